# revision 1
# baseline (speedup 1.0000x reference)
"""Trainium2 Bass kernel for DeepMinAttLSTM (4x minLSTM + MHSA + last-step FC).

Strategy:
  - Data-parallel over batch: 16 batches -> 8 cores x 2 batches.
  - Everything on device is kept feature-major: activations live as
    X^T [H=1024 (8 partition-tiles of 128), B*S=2048 free] in bf16.
  - Per layer: 3 gate matmuls (W^T stationary, X^T moving, fp32 PSUM),
    sigmoid gates on ACT, fp/add gate math on DVE (reciprocal_approx_fast),
    and the sequential minLSTM recurrence via the DVE tensor_tensor_scan
    instruction (state fp32) along the free (time) dimension.
  - The final output only uses out[:, -1, :], so attention collapses to the
    last query position: K and V are computed for the full sequence, Q/scores/
    softmax/out-proj/fc only for the last column per batch. Softmax needs no
    max-subtraction (logit absmax ~ 0.01 for this problem scale).
  - All matmuls in bf16 with fp32 accumulation (predicted rel err ~7e-3).
"""

import math

import numpy as np
import ml_dtypes

BF16 = ml_dtypes.bfloat16

P = 128
H = 1024
S = 1024
B = 16
NCORES = 8
BC = B // NCORES          # batches per core
BS = BC * S               # 2048 free columns per core
KO = H // P               # 8 feature partition-tiles
NH = 8
DH = H // NH              # 128
O = 256
L = 4
QSCALE = 1.0 / math.sqrt(DH)

_CACHE = {}


def _build_nc():
    import concourse.mybir as mybir
    import concourse.tile as tile
    from concourse import bacc

    DT = mybir.dt.bfloat16
    F32 = mybir.dt.float32
    AFT = mybir.ActivationFunctionType
    OP = mybir.AluOpType

    nc = bacc.Bacc("TRN2", target_bir_lowering=False, debug=False,
                   num_devices=NCORES)

    xT = nc.dram_tensor("xT", [P, KO * BS], DT, kind="ExternalInput").ap()
    gw = nc.dram_tensor("gw", [3 * L * P, KO * H], DT, kind="ExternalInput").ap()
    gb = nc.dram_tensor("gb", [P, 3 * L * KO], F32, kind="ExternalInput").ap()
    ip = nc.dram_tensor("ip", [P, KO * 3 * H], DT, kind="ExternalInput").ap()
    ipb = nc.dram_tensor("ipb", [P, 2 * KO], F32, kind="ExternalInput").ap()
    vb = nc.dram_tensor("vb", [P, NH], F32, kind="ExternalInput").ap()
    ow = nc.dram_tensor("ow", [P, KO * H], DT, kind="ExternalInput").ap()
    ob = nc.dram_tensor("ob", [P, KO], F32, kind="ExternalInput").ap()
    fcw = nc.dram_tensor("fcw", [P, KO * O], DT, kind="ExternalInput").ap()
    fcb = nc.dram_tensor("fcb", [P, O // P], F32, kind="ExternalInput").ap()
    outT = nc.dram_tensor("outT", [O, BC], F32, kind="ExternalOutput").ap()

    with tile.TileContext(nc) as tc:
        with (
            tc.tile_pool(name="constp", bufs=1) as constp,
            tc.tile_pool(name="hbuf", bufs=2) as hp,
        ):
            gb_sb = constp.tile([P, 3 * L * KO], F32)
            nc.sync.dma_start(gb_sb[:], gb[:])
            ones_col = constp.tile([P, 1], F32)
            nc.vector.memset(ones_col[:], 1.0)
            ones_row = constp.tile([1, P], F32)
            nc.vector.memset(ones_row[:], 1.0)

            X = hp.tile([P, KO * BS], DT, tag="hbuf", name="xT_sb")
            # chunked load so layer-0 matmuls start on the first chunk
            xT_v = xT.rearrange("p (k m) -> p k m", k=KO)
            X_v = X.rearrange("p (k m) -> p k m", k=KO)
            for ch in range(4):
                m0 = ch * 512
                nc.sync.dma_start(X_v[:, :, m0:m0 + 512],
                                  xT_v[:, :, m0:m0 + 512])

            # in_proj weights preloaded early (pool below layer pools so the
            # DMA does not wait for layer-pool release zones)
            ip_pool = tc.tile_pool(name="ipp", bufs=1)
            ipp = ip_pool.__enter__()
            ip_sb = ipp.tile([P, KO * 3 * H], DT, name="ip_sb")

            # ---------------- minLSTM layers ----------------
            with (
                tc.tile_pool(name="gwp", bufs=4) as gwp,
                tc.tile_pool(name="fpp", bufs=2) as fpp,
                tc.tile_pool(name="addp", bufs=2) as addp,
                tc.tile_pool(name="tmpp", bufs=2) as tmpp,
                tc.tile_pool(name="psA", bufs=6, space="PSUM") as psA,
            ):
                for l in range(L):
                    if l == 2:
                        # overlap the 6MB in_proj load with layers 2-3
                        nc.sync.dma_start(ip_sb[:], ip[:])
                    gws = []
                    for g in range(3):
                        lg = l * 3 + g
                        gw_t = gwp.tile([P, KO * H], DT, tag="gw",
                                        name=f"gw_{l}_{g}")
                        # gpsimd queue: runs concurrently with the xT/ip
                        # loads on the sync queue
                        nc.gpsimd.dma_start(gw_t[:],
                                            gw[lg * P:(lg + 1) * P, :])
                        gws.append(gw_t)
                    h_out = hp.tile([P, KO * BS], DT, tag="hbuf", name=f"h_{l}")
                    for no in range(KO):
                        fp_t = fpp.tile([P, BS], DT, tag="fp",
                                        name=f"fp_{l}_{no}")
                        add_t = addp.tile([P, BS], DT, tag="add",
                                          name=f"add_{l}_{no}")
                        for ch in range(4):
                            m0 = ch * 512
                            psF = psA.tile([P, 512], F32, tag="ps", name="psF")
                            psI = psA.tile([P, 512], F32, tag="ps", name="psI")
                            psH = psA.tile([P, 512], F32, tag="ps", name="psH")
                            for g, ps in ((0, psF), (1, psI), (2, psH)):
                                for ko in range(KO):
                                    nc.tensor.matmul(
                                        ps[:],
                                        gws[g][:, ko * H + no * P:
                                               ko * H + (no + 1) * P],
                                        X[:, ko * BS + m0: ko * BS + m0 + 512],
                                        start=(ko == 0), stop=(ko == KO - 1))
                            f_t = tmpp.tile([P, 512], DT, tag="f_t", name="f_t")
                            i_t = tmpp.tile([P, 512], DT, tag="i_t", name="i_t")
                            d_t = tmpp.tile([P, 512], F32, tag="d_t", name="d_t")
                            r_t = tmpp.tile([P, 512], F32, tag="r_t", name="r_t")
                            t1 = tmpp.tile([P, 512], F32, tag="t1", name="t1",
                                           bufs=1)
                            bF = gb_sb[:, (l * 3 + 0) * KO + no:
                                       (l * 3 + 0) * KO + no + 1]
                            bI = gb_sb[:, (l * 3 + 1) * KO + no:
                                       (l * 3 + 1) * KO + no + 1]
                            bH = gb_sb[:, (l * 3 + 2) * KO + no:
                                       (l * 3 + 2) * KO + no + 1]
                            nc.scalar.activation(f_t[:], psF[:], AFT.Sigmoid,
                                                 bias=bF)
                            nc.scalar.activation(i_t[:], psI[:], AFT.Sigmoid,
                                                 bias=bI)
                            nc.vector.tensor_add(d_t[:], f_t[:], i_t[:])
                            nc.vector.reciprocal_approx_fast(r_t[:], d_t[:])
                            nc.vector.tensor_mul(
                                fp_t[:, m0:m0 + 512], f_t[:], r_t[:])
                            # t1 = (ht_psum + bh) * r
                            nc.vector.scalar_tensor_tensor(
                                t1[:], psH[:], bH, r_t[:],
                                op0=OP.add, op1=OP.mult)
                            nc.vector.tensor_mul(
                                add_t[:, m0:m0 + 512], t1[:], i_t[:])
                            # recurrence piece for this chunk (b, half) —
                            # emitted here so the scan runs as soon as its
                            # inputs exist, minimizing the layer-boundary
                            # stall on the next layer's matmuls
                            b, half = ch // 2, ch % 2
                            base = no * BS + b * S
                            if half == 0:
                                nc.vector.tensor_tensor_scan(
                                    h_out[:, base: base + 512],
                                    fp_t[:, b * S: b * S + 512],
                                    add_t[:, b * S: b * S + 512],
                                    initial=0.0, op0=OP.mult, op1=OP.add)
                            else:
                                nc.vector.tensor_tensor_scan(
                                    h_out[:, base + 512: base + S],
                                    fp_t[:, b * S + 512: (b + 1) * S],
                                    add_t[:, b * S + 512: (b + 1) * S],
                                    initial=h_out[:, base + 511: base + 512],
                                    op0=OP.mult, op1=OP.add)
                    X = h_out

            h4 = X

            # ---------------- attention (last query position only) ----------
            with (
                tc.tile_pool(name="vp", bufs=1) as vp,
                tc.tile_pool(name="owp", bufs=1) as owp,
                tc.tile_pool(name="smallp", bufs=1) as smallp,
            ):
                ow_sb = owp.tile([P, KO * H], DT)
                nc.sync.dma_start(ow_sb[:], ow[:])
                fcw_sb = owp.tile([P, KO * O], DT)
                nc.sync.dma_start(fcw_sb[:], fcw[:])
                ipb_sb = constp.tile([P, 2 * KO], F32)
                nc.sync.dma_start(ipb_sb[:], ipb[:])
                vb_sb = constp.tile([P, NH], F32)
                nc.sync.dma_start(vb_sb[:], vb[:])
                ob_sb = constp.tile([P, KO], F32)
                nc.sync.dma_start(ob_sb[:], ob[:])
                fcb_sb = constp.tile([P, O // P], F32)
                nc.sync.dma_start(fcb_sb[:], fcb[:])

                K_sb = hp.tile([P, KO * BS], DT, tag="hbuf", name="K_sb")
                V_sb = vp.tile([P, KO * BS], DT, name="V_sb")
                lastq = smallp.tile([P, 2 * KO], DT)
                q_sb = smallp.tile([P, 2 * KO], DT)
                e_all = smallp.tile([P, 2 * NH * KO], DT)
                acc_all = smallp.tile([P, 2 * NH], F32)
                den_r = smallp.tile([1, 2 * NH], F32)
                rb_sb = smallp.tile([P, 2 * NH], F32)
                O_last = smallp.tile([P, 2 * KO], DT)
                out_last = smallp.tile([P, 2 * KO], DT)
                res_sb = smallp.tile([P, 2 * (O // P)], F32)

                # h4 columns at the last timestep (per ko-tile, per batch)
                for ko in range(KO):
                    for b in range(BC):
                        nc.vector.tensor_copy(
                            lastq[:, ko * BC + b: ko * BC + b + 1],
                            h4[:, ko * BS + b * S + S - 1:
                               ko * BS + b * S + S])

                with (
                    tc.tile_pool(name="psK", bufs=6, space="PSUM") as psK,
                ):
                    # K^T feature-major [H, BS]
                    for nt in range(KO):
                        for ch in range(4):
                            m0 = ch * 512
                            ps = psK.tile([P, 512], F32, tag="ps", name="psk")
                            for ko in range(KO):
                                nc.tensor.matmul(
                                    ps[:],
                                    ip_sb[:, ko * 3 * H + H + nt * P:
                                          ko * 3 * H + H + (nt + 1) * P],
                                    h4[:, ko * BS + m0: ko * BS + m0 + 512],
                                    start=(ko == 0), stop=(ko == KO - 1))
                            nc.scalar.activation(
                                K_sb[:, nt * BS + m0: nt * BS + m0 + 512],
                                ps[:], AFT.Identity,
                                bias=ipb_sb[:, KO + nt: KO + nt + 1])
                    # V position-major [BS, H]
                    for st in range(2 * KO):
                        b, si = st // KO, st % KO
                        for dch in range(2):
                            d0 = dch * 512
                            ps = psK.tile([P, 512], F32, tag="ps", name="psv")
                            for ko in range(KO):
                                nc.tensor.matmul(
                                    ps[:],
                                    h4[:, ko * BS + b * S + si * P:
                                       ko * BS + b * S + (si + 1) * P],
                                    ip_sb[:, ko * 3 * H + 2 * H + d0:
                                          ko * 3 * H + 2 * H + d0 + 512],
                                    start=(ko == 0), stop=(ko == KO - 1))
                            nc.scalar.activation(
                                V_sb[:, st * H + d0: st * H + d0 + 512],
                                ps[:], AFT.Copy)
                    # Q at the last position only (2 columns)
                    for nt in range(KO):
                        ps = psK.tile([P, 512], F32, tag="ps", name="psq")
                        for ko in range(KO):
                            nc.tensor.matmul(
                                ps[:, 0:BC],
                                ip_sb[:, ko * 3 * H + nt * P:
                                      ko * 3 * H + (nt + 1) * P],
                                lastq[:, ko * BC: (ko + 1) * BC],
                                start=(ko == 0), stop=(ko == KO - 1))
                        nc.scalar.activation(
                            q_sb[:, nt * BC: (nt + 1) * BC], ps[:, 0:BC],
                            AFT.Identity, bias=ipb_sb[:, nt: nt + 1],
                            scale=QSCALE)

                # ----- per (batch, head) tail -----
                with tc.tile_pool(name="psB", bufs=1, space="PSUM") as psB:
                    for b in range(BC):
                        for j in range(NH):
                            bj = b * NH + j
                            ps_s = psB.tile([P, KO], F32, tag="sT",
                                            name="ps_s", bufs=2)
                            for kt in range(KO):
                                nc.tensor.matmul(
                                    ps_s[:, kt:kt + 1],
                                    K_sb[:, j * BS + b * S + kt * P:
                                         j * BS + b * S + (kt + 1) * P],
                                    q_sb[:, j * BC + b: j * BC + b + 1],
                                    start=True, stop=True)
                            nc.scalar.activation(
                                e_all[:, bj * KO: (bj + 1) * KO], ps_s[:],
                                AFT.Exp,
                                accum_out=acc_all[:, bj:bj + 1])
                    # denominators: sum acc over partitions -> [1, 16]
                    ps_den = psB.tile([1, 2 * NH], F32, tag="den",
                                      name="ps_den")
                    nc.tensor.matmul(ps_den[:], ones_col[:], acc_all[:],
                                     start=True, stop=True)
                    nc.vector.reciprocal(den_r[:], ps_den[:])
                    # broadcast reciprocal across partitions -> [128, 16]
                    ps_bc = psB.tile([P, 2 * NH], F32, tag="bc", name="ps_bc")
                    nc.tensor.matmul(ps_bc[:], ones_row[:], den_r[:],
                                     start=True, stop=True)
                    nc.scalar.activation(rb_sb[:], ps_bc[:], AFT.Copy)
                    # O at last position, normalized + V bias
                    for b in range(BC):
                        for j in range(NH):
                            bj = b * NH + j
                            ps_o = psB.tile([P, 1], F32, tag="o",
                                            name="ps_o", bufs=2)
                            for kt in range(KO):
                                nc.tensor.matmul(
                                    ps_o[:],
                                    V_sb[:, (b * KO + kt) * H + j * P:
                                         (b * KO + kt) * H + (j + 1) * P],
                                    e_all[:, bj * KO + kt: bj * KO + kt + 1],
                                    start=(kt == 0), stop=(kt == KO - 1))
                            nc.vector.scalar_tensor_tensor(
                                O_last[:, j * BC + b: j * BC + b + 1],
                                ps_o[:], rb_sb[:, bj:bj + 1],
                                vb_sb[:, j:j + 1],
                                op0=OP.mult, op1=OP.add)
                    # out projection at last position + residual
                    for no in range(KO):
                        ps_p = psB.tile([P, BC], F32, tag="p", name="ps_p")
                        for ko in range(KO):
                            nc.tensor.matmul(
                                ps_p[:],
                                ow_sb[:, ko * H + no * P: ko * H + (no + 1) * P],
                                O_last[:, ko * BC: (ko + 1) * BC],
                                start=(ko == 0), stop=(ko == KO - 1))
                        nc.vector.scalar_tensor_tensor(
                            out_last[:, no * BC: (no + 1) * BC],
                            ps_p[:], ob_sb[:, no:no + 1],
                            lastq[:, no * BC: (no + 1) * BC],
                            op0=OP.add, op1=OP.add)
                    # final fc
                    for ot in range(O // P):
                        ps_f = psB.tile([P, BC], F32, tag="f", name="ps_f")
                        for ko in range(KO):
                            nc.tensor.matmul(
                                ps_f[:],
                                fcw_sb[:, ko * O + ot * P: ko * O + (ot + 1) * P],
                                out_last[:, ko * BC: (ko + 1) * BC],
                                start=(ko == 0), stop=(ko == KO - 1))
                        nc.scalar.activation(
                            res_sb[:, ot * BC: (ot + 1) * BC], ps_f[:],
                            AFT.Identity, bias=fcb_sb[:, ot:ot + 1])
                        nc.sync.dma_start(
                            outT[ot * P:(ot + 1) * P, :],
                            res_sb[:, ot * BC: (ot + 1) * BC])

            ip_pool.__exit__(None, None, None)

    nc.compile()
    return nc


def _feature_major(w_t):
    """[H_in, N] (already transposed weight) -> device layout [128, KO*N]."""
    hin, n = w_t.shape
    ko = hin // P
    return np.ascontiguousarray(
        w_t.reshape(ko, P, n).transpose(1, 0, 2).reshape(P, ko * n))


def _prep_inputs(x, Wf, bf, Wi, bi, Wh, bh, in_proj_w, in_proj_b, out_w,
                 out_b, fc_w, fc_b):
    gws = []
    gbs = []
    for l in range(L):
        for W, bias in ((Wf[l], bf[l]), (Wi[l], bi[l]), (Wh[l], bh[l])):
            gws.append(_feature_major(W.T.astype(np.float32)).astype(BF16))
            gbs.append(bias.reshape(KO, P).T.astype(np.float32))
    gw = np.concatenate(gws, axis=0)                     # [12*128, KO*H]
    gb = np.concatenate(gbs, axis=1)                     # [128, 12*KO]
    ip = _feature_major(in_proj_w.T.astype(np.float32)).astype(BF16)
    ipb = in_proj_b[:2 * H].reshape(2 * KO, P).T.astype(np.float32).copy()
    ipb[:, :KO] *= QSCALE                                # fold Q scaling
    vbv = in_proj_b[2 * H:].reshape(NH, P).T.astype(np.float32)
    owp = _feature_major(out_w.T.astype(np.float32)).astype(BF16)
    obv = out_b.reshape(KO, P).T.astype(np.float32)
    fcwp = _feature_major(fc_w.T.astype(np.float32)).astype(BF16)
    fcbv = fc_b.reshape(O // P, P).T.astype(np.float32)
    shared = dict(gw=gw, gb=np.ascontiguousarray(gb),
                  ip=ip, ipb=np.ascontiguousarray(ipb),
                  vb=np.ascontiguousarray(vbv), ow=owp,
                  ob=np.ascontiguousarray(obv), fcw=fcwp,
                  fcb=np.ascontiguousarray(fcbv))
    in_maps = []
    for c in range(NCORES):
        shard = x[c * BC:(c + 1) * BC]                   # [BC, S, H]
        xt = shard.transpose(2, 0, 1).reshape(H, BS)     # [H, BS]
        xt = _feature_major(xt).astype(BF16)             # [128, KO*BS]
        in_maps.append(dict(shared, xT=xt))
    return in_maps


def kernel(x, Wf, bf, Wi, bi, Wh, bh, in_proj_w, in_proj_b, out_w, out_b,
           fc_w, fc_b):
    from concourse.bass_utils import run_bass_kernel_spmd

    x, Wf, bf, Wi, bi, Wh, bh = (np.asarray(t) for t in
                                 (x, Wf, bf, Wi, bi, Wh, bh))
    in_proj_w, in_proj_b, out_w, out_b, fc_w, fc_b = (
        np.asarray(t) for t in (in_proj_w, in_proj_b, out_w, out_b,
                                fc_w, fc_b))
    if "nc" not in _CACHE:
        _CACHE["nc"] = _build_nc()
    nc = _CACHE["nc"]
    in_maps = _prep_inputs(x, Wf, bf, Wi, bi, Wh, bh, in_proj_w, in_proj_b,
                           out_w, out_b, fc_w, fc_b)
    res = run_bass_kernel_spmd(nc, in_maps, core_ids=list(range(NCORES)))
    _CACHE["last_results"] = res
    out = np.empty((B, O), np.float32)
    for c in range(NCORES):
        outT = res.results[c]["outT"]                    # [O, BC]
        for b in range(BC):
            out[c * BC + b] = outT[:, b]
    return out



# revision 5
# speedup vs baseline: 1.2524x; 1.2524x over previous
"""Trainium2 Bass kernel for DeepMinAttLSTM (4x minLSTM + MHSA + last-step FC).

Strategy:
  - Data-parallel over batch: 16 batches -> 8 cores x 2 batches.
  - Activations are feature-major: X^T [H=1024 (8 partition-tiles of 128),
    B*S=2048 free] in bf16; gate matmuls with W^T stationary, fp32 PSUM.
  - Gate math (per [128,512] chunk) is engine-balanced so the DVE never
    gates PSUM recycling:
      ACT : f = sigmoid(psF+bF), i = sigmoid(psI+bI), A = 1-g
      Pool: d = f + i
      DVE : r = 1/d (approx), g = i*r, B = (psH+bH)*g, scan(A,B)
    h_t = A*h_{t-1} + B  ==  (f*h + i*h~)/(f+i)   (A = f/(f+i) = 1-g)
  - Chunk loop is ch-outer so each layer finishes its time-columns in the
    order the next layer consumes them (cross-layer pipelining).
  - Attention: output only needs the last query position, so
      scores_s = q . K_s  ==  (Wk_j^T q_j) . h4_s   (per head j)
    which removes the full K matmul; K's bias shifts all scores of a query
    equally and cancels in softmax. V is computed position-major as before.
    Softmax denominators accumulate on the PE via ones-matmuls.
  - All matmuls bf16 with fp32 accumulation.
"""

import math

import numpy as np
import ml_dtypes

BF16 = ml_dtypes.bfloat16

P = 128
H = 1024
S = 1024
B = 16
NCORES = 8
BC = B // NCORES          # batches per core
BS = BC * S               # 2048 free columns per core
KO = H // P               # 8 feature partition-tiles
NH = 8
DH = H // NH              # 128
O = 256
L = 4
QSCALE = 1.0 / math.sqrt(DH)

_CACHE = {}


def _build_nc():
    import concourse.mybir as mybir
    import concourse.tile as tile
    from concourse import bacc

    DT = mybir.dt.bfloat16
    F32 = mybir.dt.float32
    AFT = mybir.ActivationFunctionType
    OP = mybir.AluOpType

    nc = bacc.Bacc("TRN2", target_bir_lowering=False, debug=False,
                   num_devices=NCORES)

    xT = nc.dram_tensor("xT", [P, KO * BS], DT, kind="ExternalInput").ap()
    gw = nc.dram_tensor("gw", [3 * L * P, KO * H], DT, kind="ExternalInput").ap()
    gb = nc.dram_tensor("gb", [P, 3 * L * KO], F32, kind="ExternalInput").ap()
    ip = nc.dram_tensor("ip", [P, KO * 3 * H], DT, kind="ExternalInput").ap()
    ipk2 = nc.dram_tensor("ipk2", [P, NH * H], DT, kind="ExternalInput").ap()
    ipb = nc.dram_tensor("ipb", [P, 2 * KO], F32, kind="ExternalInput").ap()
    vb = nc.dram_tensor("vb", [P, NH], F32, kind="ExternalInput").ap()
    ow = nc.dram_tensor("ow", [P, KO * H], DT, kind="ExternalInput").ap()
    ob = nc.dram_tensor("ob", [P, KO], F32, kind="ExternalInput").ap()
    fcw = nc.dram_tensor("fcw", [P, KO * O], DT, kind="ExternalInput").ap()
    fcb = nc.dram_tensor("fcb", [P, O // P], F32, kind="ExternalInput").ap()
    outT = nc.dram_tensor("outT", [O, BC], F32, kind="ExternalOutput").ap()

    with tile.TileContext(nc) as tc:
        with (
            tc.tile_pool(name="constp", bufs=1) as constp,
            tc.tile_pool(name="hbuf", bufs=2) as hp,
        ):
            gb_sb = constp.tile([P, 3 * L * KO], F32)
            nc.sync.dma_start(gb_sb[:], gb[:])
            ones_col = constp.tile([P, 1], DT)
            nc.vector.memset(ones_col[:], 1.0)
            ones_row = constp.tile([1, P], F32)
            nc.vector.memset(ones_row[:], 1.0)

            X = hp.tile([P, KO * BS], DT, tag="hbuf", name="xT_sb")
            # chunked load so layer-0 matmuls start on the first chunk
            xT_v = xT.rearrange("p (k m) -> p k m", k=KO)
            X_v = X.rearrange("p (k m) -> p k m", k=KO)
            for ch in range(4):
                m0 = ch * 512
                nc.sync.dma_start(X_v[:, :, m0:m0 + 512],
                                  xT_v[:, :, m0:m0 + 512])

            # in_proj weights preloaded early (pool below layer pools so the
            # DMA does not wait for layer-pool release zones)
            ip_pool = tc.tile_pool(name="ipp", bufs=1)
            ipp = ip_pool.__enter__()
            ip_sb = ipp.tile([P, KO * 3 * H], DT, name="ip_sb")
            ipk2_sb = ipp.tile([P, NH * H], DT, name="ipk2_sb")

            # ---------------- minLSTM layers ----------------
            with (
                tc.tile_pool(name="gwp", bufs=4) as gwp,
                tc.tile_pool(name="abp", bufs=4) as abp,
                tc.tile_pool(name="tmpp", bufs=2) as tmpp,
                tc.tile_pool(name="psA", bufs=6, space="PSUM") as psA,
            ):
                for l in range(L):
                    if l == 1:
                        nc.sync.dma_start(ipk2_sb[:], ipk2[:])
                    if l == 2:
                        # overlap the 6MB in_proj load with layers 2-3
                        nc.sync.dma_start(ip_sb[:], ip[:])
                    gws = []
                    for g in range(3):
                        lg = l * 3 + g
                        gw_t = gwp.tile([P, KO * H], DT, tag="gw",
                                        name=f"gw_{l}_{g}")
                        # gpsimd queue: runs concurrently with the xT/ip
                        # loads on the sync queue
                        nc.gpsimd.dma_start(gw_t[:],
                                            gw[lg * P:(lg + 1) * P, :])
                        gws.append(gw_t)
                    h_out = hp.tile([P, KO * BS], DT, tag="hbuf", name=f"h_{l}")
                    for ch in range(4):
                        m0 = ch * 512
                        b, half = ch // 2, ch % 2
                        for no in range(KO):
                            psF = psA.tile([P, 512], F32, tag="ps", name="psF")
                            psI = psA.tile([P, 512], F32, tag="ps", name="psI")
                            psH = psA.tile([P, 512], F32, tag="ps", name="psH")
                            for g, ps in ((0, psF), (1, psI), (2, psH)):
                                for ko in range(KO):
                                    nc.tensor.matmul(
                                        ps[:],
                                        gws[g][:, ko * H + no * P:
                                               ko * H + (no + 1) * P],
                                        X[:, ko * BS + m0: ko * BS + m0 + 512],
                                        start=(ko == 0), stop=(ko == KO - 1))
                            f_t = tmpp.tile([P, 512], DT, tag="f_t", name="f_t")
                            i_t = tmpp.tile([P, 512], DT, tag="i_t", name="i_t")
                            d_t = tmpp.tile([P, 512], F32, tag="d_t", name="d_t", bufs=1)
                            r_t = tmpp.tile([P, 512], F32, tag="r_t", name="r_t", bufs=1)
                            g_t = tmpp.tile([P, 512], DT, tag="g_t", name="g_t", bufs=1)
                            a_t = abp.tile([P, 512], DT, tag="ab", name="a_t")
                            b_t = abp.tile([P, 512], DT, tag="ab", name="b_t")
                            bF = gb_sb[:, (l * 3 + 0) * KO + no:
                                       (l * 3 + 0) * KO + no + 1]
                            bI = gb_sb[:, (l * 3 + 1) * KO + no:
                                       (l * 3 + 1) * KO + no + 1]
                            bH = gb_sb[:, (l * 3 + 2) * KO + no:
                                       (l * 3 + 2) * KO + no + 1]
                            nc.scalar.activation(f_t[:], psF[:], AFT.Sigmoid,
                                                 bias=bF)
                            nc.scalar.activation(i_t[:], psI[:], AFT.Sigmoid,
                                                 bias=bI)
                            # d = f + i on the (otherwise idle) gpsimd engine
                            nc.gpsimd.tensor_add(d_t[:], f_t[:], i_t[:])
                            nc.vector.reciprocal_approx_fast(r_t[:], d_t[:])
                            nc.vector.tensor_mul(g_t[:], i_t[:], r_t[:])
                            # A = 1 - g on ACT (Identity is in every table)
                            nc.scalar.activation(a_t[:], g_t[:], AFT.Identity,
                                                 bias=1.0, scale=-1.0)
                            # B = (psH + bH) * g
                            nc.vector.scalar_tensor_tensor(
                                b_t[:], psH[:], bH, g_t[:],
                                op0=OP.add, op1=OP.mult)
                            base = no * BS + b * S
                            if half == 0:
                                nc.vector.tensor_tensor_scan(
                                    h_out[:, base: base + 512],
                                    a_t[:], b_t[:],
                                    initial=0.0, op0=OP.mult, op1=OP.add)
                            else:
                                nc.vector.tensor_tensor_scan(
                                    h_out[:, base + 512: base + S],
                                    a_t[:], b_t[:],
                                    initial=h_out[:, base + 511: base + 512],
                                    op0=OP.mult, op1=OP.add)
                    X = h_out

            h4 = X

            # ---------------- attention (last query position only) ----------
            with (
                tc.tile_pool(name="vp", bufs=1) as vp,
                tc.tile_pool(name="owp", bufs=1) as owp,
                tc.tile_pool(name="smallp", bufs=1) as smallp,
            ):
                ow_sb = owp.tile([P, KO * H], DT)
                nc.sync.dma_start(ow_sb[:], ow[:])
                fcw_sb = owp.tile([P, KO * O], DT)
                nc.sync.dma_start(fcw_sb[:], fcw[:])
                ipb_sb = constp.tile([P, 2 * KO], F32)
                nc.sync.dma_start(ipb_sb[:], ipb[:])
                vb_sb = constp.tile([P, NH], F32)
                nc.sync.dma_start(vb_sb[:], vb[:])
                ob_sb = constp.tile([P, KO], F32)
                nc.sync.dma_start(ob_sb[:], ob[:])
                fcb_sb = constp.tile([P, O // P], F32)
                nc.sync.dma_start(fcb_sb[:], fcb[:])

                V_sb = vp.tile([P, KO * BS], DT, name="V_sb")
                lastq = smallp.tile([P, 2 * KO], DT)
                q_sb = smallp.tile([P, 2 * KO], DT)
                qt_sb = smallp.tile([P, KO * BC * NH], DT)   # [128, 128]
                e_all = smallp.tile([P, BC * KO * NH], DT)   # [128, 128]
                den_r = smallp.tile([1, BC * NH], F32)
                rb_sb = smallp.tile([P, BC * NH], F32)
                O_last = smallp.tile([P, 2 * KO], DT)
                out_last = smallp.tile([P, 2 * KO], DT)
                res_sb = smallp.tile([P, 2 * (O // P)], F32)

                # h4 columns at the last timestep (per ko-tile, per batch)
                for ko in range(KO):
                    for b in range(BC):
                        nc.vector.tensor_copy(
                            lastq[:, ko * BC + b: ko * BC + b + 1],
                            h4[:, ko * BS + b * S + S - 1:
                               ko * BS + b * S + S])

                with (
                    tc.tile_pool(name="psT", bufs=1, space="PSUM") as psT,
                ):
                    # q at the last position (head j occupies d-chunk j)
                    for j in range(NH):
                        psq = psT.tile([P, BC], F32, tag="sm2", name="psq",
                                       bufs=2)
                        for ko in range(KO):
                            nc.tensor.matmul(
                                psq[:],
                                ip_sb[:, ko * 3 * H + j * P:
                                      ko * 3 * H + (j + 1) * P],
                                lastq[:, ko * BC: (ko + 1) * BC],
                                start=(ko == 0), stop=(ko == KO - 1))
                        nc.scalar.activation(
                            q_sb[:, j * BC: (j + 1) * BC], psq[:],
                            AFT.Identity, bias=ipb_sb[:, j: j + 1],
                            scale=QSCALE)
                    # q~_j = Wk_j^T q_j  (folded-K scores vector)
                    qt_v = qt_sb.rearrange("p (hc b j) -> p hc b j",
                                           hc=KO, b=BC)
                    for j in range(NH):
                        psqt = psT.tile([P, KO * BC], F32, tag="w16",
                                        name="psqt", bufs=3)
                        for hc in range(KO):
                            nc.tensor.matmul(
                                psqt[:, hc * BC: (hc + 1) * BC],
                                ipk2_sb[:, j * H + hc * P:
                                        j * H + (hc + 1) * P],
                                q_sb[:, j * BC: (j + 1) * BC],
                                start=True, stop=True)
                        psqt_v = psqt.rearrange("p (hc b) -> p hc b", hc=KO)
                        nc.scalar.activation(
                            qt_v[:, :, :, j], psqt_v[:, :, :], AFT.Copy)
                    # scores via q~ . h4 (s on partitions) + exp + denom
                    ps_den_t = psT.tile([P, BC * NH], F32, tag="den",
                                        name="ps_den")
                    ps_den = ps_den_t[0:1, :]
                    for b in range(BC):
                        for kt in range(KO):
                            pss_t = psT.tile([P, KO * BC], F32, tag="w16",
                                              name="pss", bufs=3)
                            pss = pss_t[:, :NH]
                            for ko in range(KO):
                                nc.tensor.matmul(
                                    pss,
                                    h4[:, ko * BS + b * S + kt * P:
                                       ko * BS + b * S + (kt + 1) * P],
                                    qt_sb[:, ko * BC * NH + b * NH:
                                          ko * BC * NH + (b + 1) * NH],
                                    start=(ko == 0), stop=(ko == KO - 1))
                            eix = (b * KO + kt) * NH
                            nc.scalar.activation(
                                e_all[:, eix: eix + NH], pss, AFT.Exp)
                            nc.tensor.matmul(
                                ps_den[:, b * NH: (b + 1) * NH],
                                ones_col[:],
                                e_all[:, eix: eix + NH],
                                start=(kt == 0), stop=(kt == KO - 1))
                    nc.vector.reciprocal(den_r[:], ps_den)
                    # broadcast reciprocal across partitions -> [128, 16]
                    ps_bc = psT.tile([P, BC * NH], F32, tag="w16", name="ps_bc", bufs=3)
                    nc.tensor.matmul(ps_bc[:], ones_row[:], den_r[:],
                                     start=True, stop=True)
                    nc.scalar.activation(rb_sb[:], ps_bc[:], AFT.Copy)
                    # V (position-major) then e.V per batch
                    for b in range(BC):
                        for si in range(KO):
                            for dch in range(2):
                                d0 = dch * 512
                                psv = psT.tile([P, 512], F32, tag="v",
                                               name="psv", bufs=2)
                                for ko in range(KO):
                                    nc.tensor.matmul(
                                        psv[:],
                                        h4[:, ko * BS + b * S + si * P:
                                           ko * BS + b * S + (si + 1) * P],
                                        ip_sb[:, ko * 3 * H + 2 * H + d0:
                                              ko * 3 * H + 2 * H + d0 + 512],
                                        start=(ko == 0), stop=(ko == KO - 1))
                                st = b * KO + si
                                nc.scalar.activation(
                                    V_sb[:, st * H + d0: st * H + d0 + 512],
                                    psv[:], AFT.Copy)
                        for j in range(NH):
                            ps_o_t = psT.tile([P, BC], F32, tag="sm2",
                                              name="ps_o", bufs=2)
                            ps_o = ps_o_t[:, 0:1]
                            for kt in range(KO):
                                nc.tensor.matmul(
                                    ps_o,
                                    V_sb[:, (b * KO + kt) * H + j * P:
                                         (b * KO + kt) * H + (j + 1) * P],
                                    e_all[:, (b * KO + kt) * NH + j:
                                          (b * KO + kt) * NH + j + 1],
                                    start=(kt == 0), stop=(kt == KO - 1))
                            nc.vector.scalar_tensor_tensor(
                                O_last[:, j * BC + b: j * BC + b + 1],
                                ps_o, rb_sb[:, b * NH + j: b * NH + j + 1],
                                vb_sb[:, j: j + 1],
                                op0=OP.mult, op1=OP.add)
                    # out projection at last position + residual
                    for no in range(KO):
                        ps_p = psT.tile([P, BC], F32, tag="sm2", name="ps_p",
                                        bufs=2)
                        for ko in range(KO):
                            nc.tensor.matmul(
                                ps_p[:],
                                ow_sb[:, ko * H + no * P: ko * H + (no + 1) * P],
                                O_last[:, ko * BC: (ko + 1) * BC],
                                start=(ko == 0), stop=(ko == KO - 1))
                        nc.vector.scalar_tensor_tensor(
                            out_last[:, no * BC: (no + 1) * BC],
                            ps_p[:], ob_sb[:, no:no + 1],
                            lastq[:, no * BC: (no + 1) * BC],
                            op0=OP.add, op1=OP.add)
                    # final fc
                    for ot in range(O // P):
                        ps_f = psT.tile([P, BC], F32, tag="sm2", name="ps_f",
                                        bufs=2)
                        for ko in range(KO):
                            nc.tensor.matmul(
                                ps_f[:],
                                fcw_sb[:, ko * O + ot * P: ko * O + (ot + 1) * P],
                                out_last[:, ko * BC: (ko + 1) * BC],
                                start=(ko == 0), stop=(ko == KO - 1))
                        nc.scalar.activation(
                            res_sb[:, ot * BC: (ot + 1) * BC], ps_f[:],
                            AFT.Identity, bias=fcb_sb[:, ot:ot + 1])
                        nc.sync.dma_start(
                            outT[ot * P:(ot + 1) * P, :],
                            res_sb[:, ot * BC: (ot + 1) * BC])

            ip_pool.__exit__(None, None, None)

    nc.compile()
    return nc


def _feature_major(w_t):
    """[H_in, N] (already transposed weight) -> device layout [128, KO*N]."""
    hin, n = w_t.shape
    ko = hin // P
    return np.ascontiguousarray(
        w_t.reshape(ko, P, n).transpose(1, 0, 2).reshape(P, ko * n))


def _prep_inputs(x, Wf, bf, Wi, bi, Wh, bh, in_proj_w, in_proj_b, out_w,
                 out_b, fc_w, fc_b):
    gws = []
    gbs = []
    for l in range(L):
        for W, bias in ((Wf[l], bf[l]), (Wi[l], bi[l]), (Wh[l], bh[l])):
            gws.append(_feature_major(W.T.astype(np.float32)).astype(BF16))
            gbs.append(bias.reshape(KO, P).T.astype(np.float32))
    gw = np.concatenate(gws, axis=0)                     # [12*128, KO*H]
    gb = np.concatenate(gbs, axis=1)                     # [128, 12*KO]
    ip = _feature_major(in_proj_w.T.astype(np.float32)).astype(BF16)
    # Wk laid out d-major for the folded-K trick: [128 (dd), j*H + h]
    wk = in_proj_w[H:2 * H].astype(np.float32)           # [d, h]
    ipk2 = np.ascontiguousarray(
        wk.reshape(NH, DH, H).transpose(1, 0, 2).reshape(DH, NH * H)
    ).astype(BF16)
    ipb = in_proj_b[:2 * H].reshape(2 * KO, P).T.astype(np.float32).copy()
    ipb[:, :KO] *= QSCALE                                # fold Q scaling
    vbv = in_proj_b[2 * H:].reshape(NH, P).T.astype(np.float32)
    owp = _feature_major(out_w.T.astype(np.float32)).astype(BF16)
    obv = out_b.reshape(KO, P).T.astype(np.float32)
    fcwp = _feature_major(fc_w.T.astype(np.float32)).astype(BF16)
    fcbv = fc_b.reshape(O // P, P).T.astype(np.float32)
    shared = dict(gw=gw, gb=np.ascontiguousarray(gb),
                  ip=ip, ipk2=ipk2, ipb=np.ascontiguousarray(ipb),
                  vb=np.ascontiguousarray(vbv), ow=owp,
                  ob=np.ascontiguousarray(obv), fcw=fcwp,
                  fcb=np.ascontiguousarray(fcbv))
    in_maps = []
    for c in range(NCORES):
        shard = x[c * BC:(c + 1) * BC]                   # [BC, S, H]
        xt = shard.transpose(2, 0, 1).reshape(H, BS)     # [H, BS]
        xt = _feature_major(xt).astype(BF16)             # [128, KO*BS]
        in_maps.append(dict(shared, xT=xt))
    return in_maps


def kernel(x, Wf, bf, Wi, bi, Wh, bh, in_proj_w, in_proj_b, out_w, out_b,
           fc_w, fc_b):
    from concourse.bass_utils import run_bass_kernel_spmd

    x, Wf, bf, Wi, bi, Wh, bh = (np.asarray(t) for t in
                                 (x, Wf, bf, Wi, bi, Wh, bh))
    in_proj_w, in_proj_b, out_w, out_b, fc_w, fc_b = (
        np.asarray(t) for t in (in_proj_w, in_proj_b, out_w, out_b,
                                fc_w, fc_b))
    if "nc" not in _CACHE:
        _CACHE["nc"] = _build_nc()
    nc = _CACHE["nc"]
    in_maps = _prep_inputs(x, Wf, bf, Wi, bi, Wh, bh, in_proj_w, in_proj_b,
                           out_w, out_b, fc_w, fc_b)
    res = run_bass_kernel_spmd(nc, in_maps, core_ids=list(range(NCORES)))
    _CACHE["last_results"] = res
    out = np.empty((B, O), np.float32)
    for c in range(NCORES):
        outT = res.results[c]["outT"]                    # [O, BC]
        for b in range(BC):
            out[c * BC + b] = outT[:, b]
    return out


# revision 8
# speedup vs baseline: 1.2592x; 1.0054x over previous
"""Trainium2 Bass kernel for DeepMinAttLSTM (4x minLSTM + MHSA + last-step FC).

Strategy:
  - Data-parallel over batch: 16 batches -> 8 cores x 2 batches.
  - Activations are feature-major: X^T [H=1024 (8 partition-tiles of 128),
    B*S=2048 free] in bf16; gate matmuls with W^T stationary, fp32 PSUM.
  - Gate math (per [128,512] chunk) is engine-balanced so the DVE never
    gates PSUM recycling:
      ACT : f = sigmoid(psF+bF), i = sigmoid(psI+bI), A = 1-g
      Pool: d = f + i
      DVE : r = 1/d (approx), g = i*r, B = (psH+bH)*g, scan(A,B)
    h_t = A*h_{t-1} + B  ==  (f*h + i*h~)/(f+i)   (A = f/(f+i) = 1-g)
  - Chunk loop is ch-outer so each layer finishes its time-columns in the
    order the next layer consumes them (cross-layer pipelining).
  - Attention: output only needs the last query position, so
      scores_s = q . K_s  ==  (Wk_j^T q_j) . h4_s   (per head j)
    which removes the full K matmul; K's bias shifts all scores of a query
    equally and cancels in softmax. V is computed position-major as before.
    Softmax denominators accumulate on the PE via ones-matmuls.
  - All matmuls bf16 with fp32 accumulation.
"""

import math

import numpy as np
import ml_dtypes

BF16 = ml_dtypes.bfloat16

P = 128
H = 1024
S = 1024
B = 16
NCORES = 8
BC = B // NCORES          # batches per core
BS = BC * S               # 2048 free columns per core
KO = H // P               # 8 feature partition-tiles
NH = 8
DH = H // NH              # 128
O = 256
L = 4
QSCALE = 1.0 / math.sqrt(DH)

_CACHE = {}


def _build_nc():
    import concourse.mybir as mybir
    import concourse.tile as tile
    from concourse import bacc

    DT = mybir.dt.bfloat16
    F32 = mybir.dt.float32
    AFT = mybir.ActivationFunctionType
    OP = mybir.AluOpType

    nc = bacc.Bacc("TRN2", target_bir_lowering=False, debug=False,
                   num_devices=NCORES)

    xT = nc.dram_tensor("xT", [P, KO * BS], DT, kind="ExternalInput").ap()
    gw = nc.dram_tensor("gw", [3 * L * P, KO * H], DT, kind="ExternalInput").ap()
    gb = nc.dram_tensor("gb", [P, 3 * L * KO], F32, kind="ExternalInput").ap()
    ip = nc.dram_tensor("ip", [P, KO * 3 * H], DT, kind="ExternalInput").ap()
    ipk2 = nc.dram_tensor("ipk2", [P, NH * H], DT, kind="ExternalInput").ap()
    ipb = nc.dram_tensor("ipb", [P, 2 * KO], F32, kind="ExternalInput").ap()
    vb = nc.dram_tensor("vb", [P, NH], F32, kind="ExternalInput").ap()
    ow = nc.dram_tensor("ow", [P, KO * H], DT, kind="ExternalInput").ap()
    ob = nc.dram_tensor("ob", [P, KO], F32, kind="ExternalInput").ap()
    fcw = nc.dram_tensor("fcw", [P, KO * O], DT, kind="ExternalInput").ap()
    fcb = nc.dram_tensor("fcb", [P, O // P], F32, kind="ExternalInput").ap()
    outT = nc.dram_tensor("outT", [O, BC], F32, kind="ExternalOutput").ap()

    with tile.TileContext(nc) as tc:
        with (
            tc.tile_pool(name="constp", bufs=1) as constp,
            tc.tile_pool(name="hbuf", bufs=2) as hp,
        ):
            gb_sb = constp.tile([P, 3 * L * KO], F32)
            nc.sync.dma_start(gb_sb[:], gb[:])
            ones_col = constp.tile([P, 1], DT)
            nc.vector.memset(ones_col[:], 1.0)
            ones_row = constp.tile([1, P], F32)
            nc.vector.memset(ones_row[:], 1.0)

            X = hp.tile([P, KO * BS], DT, tag="hbuf", name="xT_sb")
            # chunked load so layer-0 matmuls start on the first chunk
            xT_v = xT.rearrange("p (k m) -> p k m", k=KO)
            X_v = X.rearrange("p (k m) -> p k m", k=KO)
            for ch in range(4):
                m0 = ch * 512
                nc.sync.dma_start(X_v[:, :, m0:m0 + 512],
                                  xT_v[:, :, m0:m0 + 512])

            # in_proj weights preloaded early (pool below layer pools so the
            # DMA does not wait for layer-pool release zones)
            ip_pool = tc.tile_pool(name="ipp", bufs=1)
            ipp = ip_pool.__enter__()
            ip_sb = ipp.tile([P, KO * 3 * H], DT, name="ip_sb")

            # ---------------- minLSTM layers ----------------
            with (
                tc.tile_pool(name="gwp", bufs=5) as gwp,
                tc.tile_pool(name="abp", bufs=3) as abp,
                tc.tile_pool(name="tmpp", bufs=2) as tmpp,
                tc.tile_pool(name="psA", bufs=6, space="PSUM") as psA,
            ):
                for l in range(L):
                    if l == 3:
                        # overlap the 6MB in_proj load with the last layer
                        nc.sync.dma_start(ip_sb[:], ip[:])
                    gws = []
                    for g in range(3):
                        lg = l * 3 + g
                        gw_t = gwp.tile([P, KO * H], DT, tag="gw",
                                        name=f"gw_{l}_{g}")
                        # split across the gpsimd and sync DMA queues so two
                        # gate-weight tiles stream in parallel
                        eng = nc.gpsimd if g != 1 else nc.sync
                        eng.dma_start(gw_t[:],
                                      gw[lg * P:(lg + 1) * P, :])
                        gws.append(gw_t)
                    h_out = hp.tile([P, KO * BS], DT, tag="hbuf", name=f"h_{l}")
                    for ch in range(4):
                        m0 = ch * 512
                        b, half = ch // 2, ch % 2
                        for no in range(KO):
                            psF = psA.tile([P, 512], F32, tag="ps", name="psF")
                            psI = psA.tile([P, 512], F32, tag="ps", name="psI")
                            psH = psA.tile([P, 512], F32, tag="ps", name="psH")
                            for g, ps in ((0, psF), (1, psI), (2, psH)):
                                for ko in range(KO):
                                    nc.tensor.matmul(
                                        ps[:],
                                        gws[g][:, ko * H + no * P:
                                               ko * H + (no + 1) * P],
                                        X[:, ko * BS + m0: ko * BS + m0 + 512],
                                        start=(ko == 0), stop=(ko == KO - 1))
                            f_t = tmpp.tile([P, 512], DT, tag="f_t", name="f_t")
                            i_t = tmpp.tile([P, 512], DT, tag="i_t", name="i_t")
                            d_t = tmpp.tile([P, 512], F32, tag="d_t", name="d_t", bufs=1)
                            r_t = tmpp.tile([P, 512], F32, tag="r_t", name="r_t", bufs=1)
                            g_t = tmpp.tile([P, 512], DT, tag="g_t", name="g_t", bufs=1)
                            a_t = abp.tile([P, 512], DT, tag="ab", name="a_t")
                            b_t = abp.tile([P, 512], DT, tag="ab", name="b_t")
                            bF = gb_sb[:, (l * 3 + 0) * KO + no:
                                       (l * 3 + 0) * KO + no + 1]
                            bI = gb_sb[:, (l * 3 + 1) * KO + no:
                                       (l * 3 + 1) * KO + no + 1]
                            bH = gb_sb[:, (l * 3 + 2) * KO + no:
                                       (l * 3 + 2) * KO + no + 1]
                            nc.scalar.activation(f_t[:], psF[:], AFT.Sigmoid,
                                                 bias=bF)
                            nc.scalar.activation(i_t[:], psI[:], AFT.Sigmoid,
                                                 bias=bI)
                            nc.vector.tensor_add(d_t[:], f_t[:], i_t[:])
                            nc.vector.reciprocal_approx_fast(r_t[:], d_t[:])
                            nc.vector.tensor_mul(g_t[:], i_t[:], r_t[:])
                            # A = 1 - g (single-src DVE op runs in 4x mode)
                            nc.vector.tensor_scalar(
                                a_t[:], g_t[:], -1.0, 1.0,
                                op0=OP.mult, op1=OP.add)
                            # B = (psH + bH) * g
                            nc.vector.scalar_tensor_tensor(
                                b_t[:], psH[:], bH, g_t[:],
                                op0=OP.add, op1=OP.mult)
                            base = no * BS + b * S
                            if half == 0:
                                nc.vector.tensor_tensor_scan(
                                    h_out[:, base: base + 512],
                                    a_t[:], b_t[:],
                                    initial=0.0, op0=OP.mult, op1=OP.add)
                            else:
                                nc.vector.tensor_tensor_scan(
                                    h_out[:, base + 512: base + S],
                                    a_t[:], b_t[:],
                                    initial=h_out[:, base + 511: base + 512],
                                    op0=OP.mult, op1=OP.add)
                    X = h_out

            h4 = X

            # ---------------- attention (last query position only) ----------
            with (
                tc.tile_pool(name="vp", bufs=1) as vp,
                tc.tile_pool(name="owp", bufs=1) as owp,
                tc.tile_pool(name="smallp", bufs=1) as smallp,
            ):
                ow_sb = owp.tile([P, KO * H], DT)
                nc.sync.dma_start(ow_sb[:], ow[:])
                fcw_sb = owp.tile([P, KO * O], DT)
                nc.sync.dma_start(fcw_sb[:], fcw[:])
                ipb_sb = constp.tile([P, 2 * KO], F32)
                nc.sync.dma_start(ipb_sb[:], ipb[:])
                vb_sb = constp.tile([P, NH], F32)
                nc.sync.dma_start(vb_sb[:], vb[:])
                ob_sb = constp.tile([P, KO], F32)
                nc.sync.dma_start(ob_sb[:], ob[:])
                fcb_sb = constp.tile([P, O // P], F32)
                nc.sync.dma_start(fcb_sb[:], fcb[:])

                V_sb = vp.tile([P, KO * BS], DT, name="V_sb")
                ipk2_sb = vp.tile([P, NH * H], DT, name="ipk2_sb")
                nc.sync.dma_start(ipk2_sb[:], ipk2[:])
                lastq = smallp.tile([P, 2 * KO], DT)
                q_sb = smallp.tile([P, 2 * KO], DT)
                qt_sb = smallp.tile([P, KO * BC * NH], DT)   # [128, 128]
                e_all = smallp.tile([P, BC * KO * NH], DT)   # [128, 128]
                den_r = smallp.tile([1, BC * NH], F32)
                rb_sb = smallp.tile([P, BC * NH], F32)
                O_last = smallp.tile([P, 2 * KO], DT)
                out_last = smallp.tile([P, 2 * KO], DT)
                res_sb = smallp.tile([P, 2 * (O // P)], F32)

                # h4 columns at the last timestep (per ko-tile, per batch)
                for ko in range(KO):
                    for b in range(BC):
                        nc.vector.tensor_copy(
                            lastq[:, ko * BC + b: ko * BC + b + 1],
                            h4[:, ko * BS + b * S + S - 1:
                               ko * BS + b * S + S])

                with (
                    tc.tile_pool(name="psT", bufs=1, space="PSUM") as psT,
                ):
                    # q at the last position (head j occupies d-chunk j)
                    for j in range(NH):
                        psq = psT.tile([P, BC], F32, tag="sm2", name="psq",
                                       bufs=2)
                        for ko in range(KO):
                            nc.tensor.matmul(
                                psq[:],
                                ip_sb[:, ko * 3 * H + j * P:
                                      ko * 3 * H + (j + 1) * P],
                                lastq[:, ko * BC: (ko + 1) * BC],
                                start=(ko == 0), stop=(ko == KO - 1))
                        nc.scalar.activation(
                            q_sb[:, j * BC: (j + 1) * BC], psq[:],
                            AFT.Identity, bias=ipb_sb[:, j: j + 1],
                            scale=QSCALE)
                    # q~_j = Wk_j^T q_j  (folded-K scores vector)
                    qt_v = qt_sb.rearrange("p (hc b j) -> p hc b j",
                                           hc=KO, b=BC)
                    for j in range(NH):
                        psqt = psT.tile([P, KO * BC], F32, tag="w16",
                                        name="psqt", bufs=3)
                        for hc in range(KO):
                            nc.tensor.matmul(
                                psqt[:, hc * BC: (hc + 1) * BC],
                                ipk2_sb[:, j * H + hc * P:
                                        j * H + (hc + 1) * P],
                                q_sb[:, j * BC: (j + 1) * BC],
                                start=True, stop=True)
                        psqt_v = psqt.rearrange("p (hc b) -> p hc b", hc=KO)
                        nc.scalar.activation(
                            qt_v[:, :, :, j], psqt_v[:, :, :], AFT.Copy)
                    # scores via q~ . h4 (s on partitions) + exp + denom
                    ps_den_t = psT.tile([P, BC * NH], F32, tag="den",
                                        name="ps_den")
                    ps_den = ps_den_t[0:1, :]
                    for b in range(BC):
                        for kt in range(KO):
                            pss_t = psT.tile([P, KO * BC], F32, tag="w16",
                                              name="pss", bufs=3)
                            pss = pss_t[:, :NH]
                            for ko in range(KO):
                                nc.tensor.matmul(
                                    pss,
                                    h4[:, ko * BS + b * S + kt * P:
                                       ko * BS + b * S + (kt + 1) * P],
                                    qt_sb[:, ko * BC * NH + b * NH:
                                          ko * BC * NH + (b + 1) * NH],
                                    start=(ko == 0), stop=(ko == KO - 1))
                            eix = (b * KO + kt) * NH
                            nc.scalar.activation(
                                e_all[:, eix: eix + NH], pss, AFT.Exp)
                            nc.tensor.matmul(
                                ps_den[:, b * NH: (b + 1) * NH],
                                ones_col[:],
                                e_all[:, eix: eix + NH],
                                start=(kt == 0), stop=(kt == KO - 1))
                    nc.vector.reciprocal(den_r[:], ps_den)
                    # broadcast reciprocal across partitions -> [128, 16]
                    ps_bc = psT.tile([P, BC * NH], F32, tag="w16", name="ps_bc", bufs=3)
                    nc.tensor.matmul(ps_bc[:], ones_row[:], den_r[:],
                                     start=True, stop=True)
                    nc.scalar.activation(rb_sb[:], ps_bc[:], AFT.Copy)
                    # V (position-major) then e.V per batch
                    for b in range(BC):
                        for si in range(KO):
                            for dch in range(2):
                                d0 = dch * 512
                                psv = psT.tile([P, 512], F32, tag="v",
                                               name="psv", bufs=2)
                                for ko in range(KO):
                                    nc.tensor.matmul(
                                        psv[:],
                                        h4[:, ko * BS + b * S + si * P:
                                           ko * BS + b * S + (si + 1) * P],
                                        ip_sb[:, ko * 3 * H + 2 * H + d0:
                                              ko * 3 * H + 2 * H + d0 + 512],
                                        start=(ko == 0), stop=(ko == KO - 1))
                                st = b * KO + si
                                nc.scalar.activation(
                                    V_sb[:, st * H + d0: st * H + d0 + 512],
                                    psv[:], AFT.Copy)
                        for j in range(NH):
                            ps_o_t = psT.tile([P, BC], F32, tag="sm2",
                                              name="ps_o", bufs=2)
                            ps_o = ps_o_t[:, 0:1]
                            for kt in range(KO):
                                nc.tensor.matmul(
                                    ps_o,
                                    V_sb[:, (b * KO + kt) * H + j * P:
                                         (b * KO + kt) * H + (j + 1) * P],
                                    e_all[:, (b * KO + kt) * NH + j:
                                          (b * KO + kt) * NH + j + 1],
                                    start=(kt == 0), stop=(kt == KO - 1))
                            nc.vector.scalar_tensor_tensor(
                                O_last[:, j * BC + b: j * BC + b + 1],
                                ps_o, rb_sb[:, b * NH + j: b * NH + j + 1],
                                vb_sb[:, j: j + 1],
                                op0=OP.mult, op1=OP.add)
                    # out projection at last position + residual
                    for no in range(KO):
                        ps_p = psT.tile([P, BC], F32, tag="sm2", name="ps_p",
                                        bufs=2)
                        for ko in range(KO):
                            nc.tensor.matmul(
                                ps_p[:],
                                ow_sb[:, ko * H + no * P: ko * H + (no + 1) * P],
                                O_last[:, ko * BC: (ko + 1) * BC],
                                start=(ko == 0), stop=(ko == KO - 1))
                        nc.vector.scalar_tensor_tensor(
                            out_last[:, no * BC: (no + 1) * BC],
                            ps_p[:], ob_sb[:, no:no + 1],
                            lastq[:, no * BC: (no + 1) * BC],
                            op0=OP.add, op1=OP.add)
                    # final fc
                    for ot in range(O // P):
                        ps_f = psT.tile([P, BC], F32, tag="sm2", name="ps_f",
                                        bufs=2)
                        for ko in range(KO):
                            nc.tensor.matmul(
                                ps_f[:],
                                fcw_sb[:, ko * O + ot * P: ko * O + (ot + 1) * P],
                                out_last[:, ko * BC: (ko + 1) * BC],
                                start=(ko == 0), stop=(ko == KO - 1))
                        nc.scalar.activation(
                            res_sb[:, ot * BC: (ot + 1) * BC], ps_f[:],
                            AFT.Identity, bias=fcb_sb[:, ot:ot + 1])
                        nc.sync.dma_start(
                            outT[ot * P:(ot + 1) * P, :],
                            res_sb[:, ot * BC: (ot + 1) * BC])

            ip_pool.__exit__(None, None, None)

    nc.compile()
    return nc


def _feature_major(w_t):
    """[H_in, N] (already transposed weight) -> device layout [128, KO*N]."""
    hin, n = w_t.shape
    ko = hin // P
    return np.ascontiguousarray(
        w_t.reshape(ko, P, n).transpose(1, 0, 2).reshape(P, ko * n))


def _prep_inputs(x, Wf, bf, Wi, bi, Wh, bh, in_proj_w, in_proj_b, out_w,
                 out_b, fc_w, fc_b):
    gws = []
    gbs = []
    for l in range(L):
        for W, bias in ((Wf[l], bf[l]), (Wi[l], bi[l]), (Wh[l], bh[l])):
            gws.append(_feature_major(W.T.astype(np.float32)).astype(BF16))
            gbs.append(bias.reshape(KO, P).T.astype(np.float32))
    gw = np.concatenate(gws, axis=0)                     # [12*128, KO*H]
    gb = np.concatenate(gbs, axis=1)                     # [128, 12*KO]
    ip = _feature_major(in_proj_w.T.astype(np.float32)).astype(BF16)
    # Wk laid out d-major for the folded-K trick: [128 (dd), j*H + h]
    wk = in_proj_w[H:2 * H].astype(np.float32)           # [d, h]
    ipk2 = np.ascontiguousarray(
        wk.reshape(NH, DH, H).transpose(1, 0, 2).reshape(DH, NH * H)
    ).astype(BF16)
    ipb = in_proj_b[:2 * H].reshape(2 * KO, P).T.astype(np.float32).copy()
    ipb[:, :KO] *= QSCALE                                # fold Q scaling
    vbv = in_proj_b[2 * H:].reshape(NH, P).T.astype(np.float32)
    owp = _feature_major(out_w.T.astype(np.float32)).astype(BF16)
    obv = out_b.reshape(KO, P).T.astype(np.float32)
    fcwp = _feature_major(fc_w.T.astype(np.float32)).astype(BF16)
    fcbv = fc_b.reshape(O // P, P).T.astype(np.float32)
    shared = dict(gw=gw, gb=np.ascontiguousarray(gb),
                  ip=ip, ipk2=ipk2, ipb=np.ascontiguousarray(ipb),
                  vb=np.ascontiguousarray(vbv), ow=owp,
                  ob=np.ascontiguousarray(obv), fcw=fcwp,
                  fcb=np.ascontiguousarray(fcbv))
    in_maps = []
    for c in range(NCORES):
        shard = x[c * BC:(c + 1) * BC]                   # [BC, S, H]
        xt = shard.transpose(2, 0, 1).reshape(H, BS)     # [H, BS]
        xt = _feature_major(xt).astype(BF16)             # [128, KO*BS]
        in_maps.append(dict(shared, xT=xt))
    return in_maps


def kernel(x, Wf, bf, Wi, bi, Wh, bh, in_proj_w, in_proj_b, out_w, out_b,
           fc_w, fc_b):
    from concourse.bass_utils import run_bass_kernel_spmd

    x, Wf, bf, Wi, bi, Wh, bh = (np.asarray(t) for t in
                                 (x, Wf, bf, Wi, bi, Wh, bh))
    in_proj_w, in_proj_b, out_w, out_b, fc_w, fc_b = (
        np.asarray(t) for t in (in_proj_w, in_proj_b, out_w, out_b,
                                fc_w, fc_b))
    if "nc" not in _CACHE:
        _CACHE["nc"] = _build_nc()
    nc = _CACHE["nc"]
    in_maps = _prep_inputs(x, Wf, bf, Wi, bi, Wh, bh, in_proj_w, in_proj_b,
                           out_w, out_b, fc_w, fc_b)
    res = run_bass_kernel_spmd(nc, in_maps, core_ids=list(range(NCORES)))
    _CACHE["last_results"] = res
    out = np.empty((B, O), np.float32)
    for c in range(NCORES):
        outT = res.results[c]["outT"]                    # [O, BC]
        for b in range(BC):
            out[c * BC + b] = outT[:, b]
    return out


# revision 11
# speedup vs baseline: 1.2719x; 1.0100x over previous
"""Trainium2 Bass kernel for DeepMinAttLSTM (4x minLSTM + MHSA + last-step FC).

Strategy:
  - Data-parallel over batch: 16 batches -> 8 cores x 2 batches.
  - Activations are feature-major: X^T [H=1024 (8 partition-tiles of 128),
    B*S=2048 free] in bf16; gate matmuls with W^T stationary, fp32 PSUM.
  - Gate math (per [128,512] chunk) is engine-balanced so the DVE never
    gates PSUM recycling:
      ACT : f = sigmoid(psF+bF), i = sigmoid(psI+bI), A = 1-g
      Pool: d = f + i
      DVE : r = 1/d (approx), g = i*r, B = (psH+bH)*g, scan(A,B)
    h_t = A*h_{t-1} + B  ==  (f*h + i*h~)/(f+i)   (A = f/(f+i) = 1-g)
  - Chunk loop is ch-outer so each layer finishes its time-columns in the
    order the next layer consumes them (cross-layer pipelining).
  - Attention: output only needs the last query position, so
      scores_s = q . K_s  ==  (Wk_j^T q_j) . h4_s   (per head j)
    which removes the full K matmul; K's bias shifts all scores of a query
    equally and cancels in softmax. V is computed position-major as before.
    Softmax denominators accumulate on the PE via ones-matmuls.
  - All matmuls bf16 with fp32 accumulation.
"""

import math

import numpy as np
import ml_dtypes

BF16 = ml_dtypes.bfloat16

P = 128
H = 1024
S = 1024
B = 16
NCORES = 8
BC = B // NCORES          # batches per core
BS = BC * S               # 2048 free columns per core
KO = H // P               # 8 feature partition-tiles
NH = 8
DH = H // NH              # 128
O = 256
L = 4
QSCALE = 1.0 / math.sqrt(DH)

_CACHE = {}


def _build_nc():
    import concourse.mybir as mybir
    import concourse.tile as tile
    from concourse import bacc

    DT = mybir.dt.bfloat16
    F32 = mybir.dt.float32
    AFT = mybir.ActivationFunctionType
    OP = mybir.AluOpType

    nc = bacc.Bacc("TRN2", target_bir_lowering=False, debug=False,
                   num_devices=NCORES)

    xT = nc.dram_tensor("xT", [P, KO * BS], DT, kind="ExternalInput").ap()
    gw = nc.dram_tensor("gw", [3 * L * P, KO * H], DT, kind="ExternalInput").ap()
    gb = nc.dram_tensor("gb", [P, 3 * L * KO], F32, kind="ExternalInput").ap()
    ip = nc.dram_tensor("ip", [P, KO * 3 * H], DT, kind="ExternalInput").ap()
    ipk2 = nc.dram_tensor("ipk2", [P, NH * H], DT, kind="ExternalInput").ap()
    ipb = nc.dram_tensor("ipb", [P, 2 * KO], F32, kind="ExternalInput").ap()
    vb = nc.dram_tensor("vb", [P, NH], F32, kind="ExternalInput").ap()
    ow = nc.dram_tensor("ow", [P, KO * H], DT, kind="ExternalInput").ap()
    ob = nc.dram_tensor("ob", [P, KO], F32, kind="ExternalInput").ap()
    fcw = nc.dram_tensor("fcw", [P, KO * O], DT, kind="ExternalInput").ap()
    fcb = nc.dram_tensor("fcb", [P, O // P], F32, kind="ExternalInput").ap()
    outT = nc.dram_tensor("outT", [O, BC], F32, kind="ExternalOutput").ap()

    with tile.TileContext(nc) as tc:
        with (
            tc.tile_pool(name="constp", bufs=1) as constp,
            tc.tile_pool(name="hbuf", bufs=2) as hp,
        ):
            gb_sb = constp.tile([P, 3 * L * KO], F32)
            nc.sync.dma_start(gb_sb[:], gb[:])
            ones_col = constp.tile([P, 1], DT)
            nc.vector.memset(ones_col[:], 1.0)
            ones_row = constp.tile([1, P], F32)
            nc.vector.memset(ones_row[:], 1.0)

            X = hp.tile([P, KO * BS], DT, tag="hbuf", name="xT_sb")
            # chunked load so layer-0 matmuls start on the first chunk;
            # chunks 1-3 are emitted after layer-0's weight DMAs (queue order)
            xT_v = xT.rearrange("p (k m) -> p k m", k=KO)
            X_v = X.rearrange("p (k m) -> p k m", k=KO)
            nc.sync.dma_start(X_v[:, :, 0:512], xT_v[:, :, 0:512])

            # in_proj weights preloaded early (pool below layer pools so the
            # DMA does not wait for layer-pool release zones)
            ip_pool = tc.tile_pool(name="ipp", bufs=1)
            ipp = ip_pool.__enter__()
            ip_sb = ipp.tile([P, KO * 3 * H], DT, name="ip_sb")

            # ---------------- minLSTM layers ----------------
            with (
                tc.tile_pool(name="gwp", bufs=10) as gwp,
                tc.tile_pool(name="abp", bufs=3) as abp,
                tc.tile_pool(name="tmpp", bufs=2) as tmpp,
                tc.tile_pool(name="psA", bufs=6, space="PSUM") as psA,
            ):
                for l in range(L):
                    if l == 3:
                        # overlap the 6MB in_proj load with the last layer
                        nc.sync.dma_start(ip_sb[:], ip[:])
                    gws = []
                    for g in range(3):
                        lg = l * 3 + g
                        halves = []
                        for hk in range(2):
                            gw_t = gwp.tile([P, KO * H // 2], DT, tag="gw",
                                            name=f"gw_{l}_{g}_{hk}")
                            # halves stream on different DMA queues so the
                            # late-freed slot refills in ~3us, not ~7us
                            eng = nc.gpsimd if hk == 0 else nc.sync
                            eng.dma_start(
                                gw_t[:],
                                gw[lg * P:(lg + 1) * P,
                                   hk * (KO * H // 2):(hk + 1) * (KO * H // 2)])
                            halves.append(gw_t)
                        gws.append(halves)
                    if l == 0:
                        for xch in range(1, 4):
                            xm = xch * 512
                            nc.sync.dma_start(X_v[:, :, xm:xm + 512],
                                              xT_v[:, :, xm:xm + 512])
                    h_out = hp.tile([P, KO * BS], DT, tag="hbuf", name=f"h_{l}")
                    for ch in range(4):
                        m0 = ch * 512
                        b, half = ch // 2, ch % 2
                        for no in range(KO):
                            psF = psA.tile([P, 512], F32, tag="ps", name="psF")
                            psI = psA.tile([P, 512], F32, tag="ps", name="psI")
                            psH = psA.tile([P, 512], F32, tag="ps", name="psH")
                            for g, ps in ((0, psF), (1, psI), (2, psH)):
                                for ko in range(KO):
                                    wt = gws[g][ko // 4]
                                    kk = ko % 4
                                    nc.tensor.matmul(
                                        ps[:],
                                        wt[:, kk * H + no * P:
                                           kk * H + (no + 1) * P],
                                        X[:, ko * BS + m0: ko * BS + m0 + 512],
                                        start=(ko == 0), stop=(ko == KO - 1))
                            f_t = tmpp.tile([P, 512], DT, tag="f_t", name="f_t")
                            i_t = tmpp.tile([P, 512], DT, tag="i_t", name="i_t")
                            d_t = tmpp.tile([P, 512], F32, tag="d_t", name="d_t", bufs=1)
                            r_t = tmpp.tile([P, 512], F32, tag="r_t", name="r_t", bufs=1)
                            g_t = tmpp.tile([P, 512], DT, tag="g_t", name="g_t", bufs=1)
                            a_t = abp.tile([P, 512], DT, tag="ab", name="a_t")
                            b_t = abp.tile([P, 512], DT, tag="ab", name="b_t")
                            bF = gb_sb[:, (l * 3 + 0) * KO + no:
                                       (l * 3 + 0) * KO + no + 1]
                            bI = gb_sb[:, (l * 3 + 1) * KO + no:
                                       (l * 3 + 1) * KO + no + 1]
                            bH = gb_sb[:, (l * 3 + 2) * KO + no:
                                       (l * 3 + 2) * KO + no + 1]
                            nc.scalar.activation(f_t[:], psF[:], AFT.Sigmoid,
                                                 bias=bF)
                            nc.scalar.activation(i_t[:], psI[:], AFT.Sigmoid,
                                                 bias=bI)
                            nc.vector.tensor_add(d_t[:], f_t[:], i_t[:])
                            nc.vector.reciprocal_approx_fast(r_t[:], d_t[:])
                            nc.vector.tensor_mul(g_t[:], i_t[:], r_t[:])
                            # A = 1 - g (single-src DVE op runs in 4x mode)
                            nc.vector.tensor_scalar(
                                a_t[:], g_t[:], -1.0, 1.0,
                                op0=OP.mult, op1=OP.add)
                            # B = (psH + bH) * g
                            nc.vector.scalar_tensor_tensor(
                                b_t[:], psH[:], bH, g_t[:],
                                op0=OP.add, op1=OP.mult)
                            base = no * BS + b * S
                            if half == 0:
                                nc.vector.tensor_tensor_scan(
                                    h_out[:, base: base + 512],
                                    a_t[:], b_t[:],
                                    initial=0.0, op0=OP.mult, op1=OP.add)
                            else:
                                nc.vector.tensor_tensor_scan(
                                    h_out[:, base + 512: base + S],
                                    a_t[:], b_t[:],
                                    initial=h_out[:, base + 511: base + 512],
                                    op0=OP.mult, op1=OP.add)
                    X = h_out

            h4 = X

            # ---------------- attention (last query position only) ----------
            with (
                tc.tile_pool(name="vp", bufs=1) as vp,
                tc.tile_pool(name="owp", bufs=1) as owp,
                tc.tile_pool(name="smallp", bufs=1) as smallp,
            ):
                ow_sb = owp.tile([P, KO * H], DT)
                nc.sync.dma_start(ow_sb[:], ow[:])
                fcw_sb = owp.tile([P, KO * O], DT)
                nc.sync.dma_start(fcw_sb[:], fcw[:])
                ipb_sb = constp.tile([P, 2 * KO], F32)
                nc.sync.dma_start(ipb_sb[:], ipb[:])
                vb_sb = constp.tile([P, NH], F32)
                nc.sync.dma_start(vb_sb[:], vb[:])
                ob_sb = constp.tile([P, KO], F32)
                nc.sync.dma_start(ob_sb[:], ob[:])
                fcb_sb = constp.tile([P, O // P], F32)
                nc.sync.dma_start(fcb_sb[:], fcb[:])

                V_sb = vp.tile([P, KO * BS], DT, name="V_sb")
                ipk2_sb = vp.tile([P, NH * H], DT, name="ipk2_sb")
                nc.sync.dma_start(ipk2_sb[:], ipk2[:])
                lastq = smallp.tile([P, 2 * KO], DT)
                q_sb = smallp.tile([P, 2 * KO], DT)
                qt_sb = smallp.tile([P, KO * BC * NH], DT)   # [128, 128]
                e_all = smallp.tile([P, BC * KO * NH], DT)   # [128, 128]
                den_r = smallp.tile([1, BC * NH], F32)
                rb_sb = smallp.tile([P, BC * NH], F32)
                O_last = smallp.tile([P, 2 * KO], DT)
                out_last = smallp.tile([P, 2 * KO], DT)
                res_sb = smallp.tile([P, 2 * (O // P)], F32)

                # h4 columns at the last timestep (per ko-tile, per batch)
                for ko in range(KO):
                    for b in range(BC):
                        nc.vector.tensor_copy(
                            lastq[:, ko * BC + b: ko * BC + b + 1],
                            h4[:, ko * BS + b * S + S - 1:
                               ko * BS + b * S + S])

                with (
                    tc.tile_pool(name="psT", bufs=1, space="PSUM") as psT,
                ):
                    def emit_V(b):
                        # V (position-major) for batch b
                        for si in range(KO):
                            for dch in range(2):
                                d0 = dch * 512
                                psv = psT.tile([P, 512], F32, tag="v",
                                               name="psv", bufs=2)
                                for ko in range(KO):
                                    nc.tensor.matmul(
                                        psv[:],
                                        h4[:, ko * BS + b * S + si * P:
                                           ko * BS + b * S + (si + 1) * P],
                                        ip_sb[:, ko * 3 * H + 2 * H + d0:
                                              ko * 3 * H + 2 * H + d0 + 512],
                                        start=(ko == 0), stop=(ko == KO - 1))
                                st = b * KO + si
                                nc.scalar.activation(
                                    V_sb[:, st * H + d0: st * H + d0 + 512],
                                    psv[:], AFT.Copy)

                    def emit_eV(b):
                        for j in range(NH):
                            ps_o_t = psT.tile([P, BC], F32, tag="sm2",
                                              name="ps_o", bufs=2)
                            ps_o = ps_o_t[:, 0:1]
                            for kt in range(KO):
                                nc.tensor.matmul(
                                    ps_o,
                                    V_sb[:, (b * KO + kt) * H + j * P:
                                         (b * KO + kt) * H + (j + 1) * P],
                                    e_all[:, (b * KO + kt) * NH + j:
                                          (b * KO + kt) * NH + j + 1],
                                    start=(kt == 0), stop=(kt == KO - 1))
                            nc.vector.scalar_tensor_tensor(
                                O_last[:, j * BC + b: j * BC + b + 1],
                                ps_o, rb_sb[:, b * NH + j: b * NH + j + 1],
                                vb_sb[:, j: j + 1],
                                op0=OP.mult, op1=OP.add)

                    # V(b=0) first: fills the PE while the last layer's scan
                    # chain and lastq drain
                    emit_V(0)
                    # q at the last position (head j occupies d-chunk j)
                    for j in range(NH):
                        psq = psT.tile([P, BC], F32, tag="sm2", name="psq",
                                       bufs=2)
                        for ko in range(KO):
                            nc.tensor.matmul(
                                psq[:],
                                ip_sb[:, ko * 3 * H + j * P:
                                      ko * 3 * H + (j + 1) * P],
                                lastq[:, ko * BC: (ko + 1) * BC],
                                start=(ko == 0), stop=(ko == KO - 1))
                        nc.scalar.activation(
                            q_sb[:, j * BC: (j + 1) * BC], psq[:],
                            AFT.Identity, bias=ipb_sb[:, j: j + 1],
                            scale=QSCALE)
                    # q~_j = Wk_j^T q_j  (folded-K scores vector)
                    qt_v = qt_sb.rearrange("p (hc b j) -> p hc b j",
                                           hc=KO, b=BC)
                    for j in range(NH):
                        psqt = psT.tile([P, KO * BC], F32, tag="w16",
                                        name="psqt", bufs=3)
                        for hc in range(KO):
                            nc.tensor.matmul(
                                psqt[:, hc * BC: (hc + 1) * BC],
                                ipk2_sb[:, j * H + hc * P:
                                        j * H + (hc + 1) * P],
                                q_sb[:, j * BC: (j + 1) * BC],
                                start=True, stop=True)
                        psqt_v = psqt.rearrange("p (hc b) -> p hc b", hc=KO)
                        nc.scalar.activation(
                            qt_v[:, :, :, j], psqt_v[:, :, :], AFT.Copy)
                    # scores via q~ . h4 (s on partitions) + exp + denom
                    ps_den_t = psT.tile([P, BC * NH], F32, tag="den",
                                        name="ps_den")
                    ps_den = ps_den_t[0:1, :]
                    for b in range(BC):
                        for kt in range(KO):
                            pss_t = psT.tile([P, KO * BC], F32, tag="w16",
                                              name="pss", bufs=3)
                            pss = pss_t[:, :NH]
                            for ko in range(KO):
                                nc.tensor.matmul(
                                    pss,
                                    h4[:, ko * BS + b * S + kt * P:
                                       ko * BS + b * S + (kt + 1) * P],
                                    qt_sb[:, ko * BC * NH + b * NH:
                                          ko * BC * NH + (b + 1) * NH],
                                    start=(ko == 0), stop=(ko == KO - 1))
                            eix = (b * KO + kt) * NH
                            nc.scalar.activation(
                                e_all[:, eix: eix + NH], pss, AFT.Exp)
                            nc.tensor.matmul(
                                ps_den[:, b * NH: (b + 1) * NH],
                                ones_col[:],
                                e_all[:, eix: eix + NH],
                                start=(kt == 0), stop=(kt == KO - 1))
                    nc.vector.reciprocal(den_r[:], ps_den)
                    # broadcast reciprocal across partitions -> [128, 16]
                    ps_bc = psT.tile([P, BC * NH], F32, tag="w16", name="ps_bc", bufs=3)
                    nc.tensor.matmul(ps_bc[:], ones_row[:], den_r[:],
                                     start=True, stop=True)
                    nc.scalar.activation(rb_sb[:], ps_bc[:], AFT.Copy)
                    emit_V(1)
                    emit_eV(0)
                    emit_eV(1)
                    # out projection at last position + residual
                    for no in range(KO):
                        ps_p = psT.tile([P, BC], F32, tag="sm2", name="ps_p",
                                        bufs=2)
                        for ko in range(KO):
                            nc.tensor.matmul(
                                ps_p[:],
                                ow_sb[:, ko * H + no * P: ko * H + (no + 1) * P],
                                O_last[:, ko * BC: (ko + 1) * BC],
                                start=(ko == 0), stop=(ko == KO - 1))
                        nc.vector.scalar_tensor_tensor(
                            out_last[:, no * BC: (no + 1) * BC],
                            ps_p[:], ob_sb[:, no:no + 1],
                            lastq[:, no * BC: (no + 1) * BC],
                            op0=OP.add, op1=OP.add)
                    # final fc
                    for ot in range(O // P):
                        ps_f = psT.tile([P, BC], F32, tag="sm2", name="ps_f",
                                        bufs=2)
                        for ko in range(KO):
                            nc.tensor.matmul(
                                ps_f[:],
                                fcw_sb[:, ko * O + ot * P: ko * O + (ot + 1) * P],
                                out_last[:, ko * BC: (ko + 1) * BC],
                                start=(ko == 0), stop=(ko == KO - 1))
                        nc.scalar.activation(
                            res_sb[:, ot * BC: (ot + 1) * BC], ps_f[:],
                            AFT.Identity, bias=fcb_sb[:, ot:ot + 1])
                        nc.sync.dma_start(
                            outT[ot * P:(ot + 1) * P, :],
                            res_sb[:, ot * BC: (ot + 1) * BC])

            ip_pool.__exit__(None, None, None)

    nc.compile()
    return nc


def _feature_major(w_t):
    """[H_in, N] (already transposed weight) -> device layout [128, KO*N]."""
    hin, n = w_t.shape
    ko = hin // P
    return np.ascontiguousarray(
        w_t.reshape(ko, P, n).transpose(1, 0, 2).reshape(P, ko * n))


def _prep_inputs(x, Wf, bf, Wi, bi, Wh, bh, in_proj_w, in_proj_b, out_w,
                 out_b, fc_w, fc_b):
    gws = []
    gbs = []
    for l in range(L):
        for W, bias in ((Wf[l], bf[l]), (Wi[l], bi[l]), (Wh[l], bh[l])):
            gws.append(_feature_major(W.T.astype(np.float32)).astype(BF16))
            gbs.append(bias.reshape(KO, P).T.astype(np.float32))
    gw = np.concatenate(gws, axis=0)                     # [12*128, KO*H]
    gb = np.concatenate(gbs, axis=1)                     # [128, 12*KO]
    ip = _feature_major(in_proj_w.T.astype(np.float32)).astype(BF16)
    # Wk laid out d-major for the folded-K trick: [128 (dd), j*H + h]
    wk = in_proj_w[H:2 * H].astype(np.float32)           # [d, h]
    ipk2 = np.ascontiguousarray(
        wk.reshape(NH, DH, H).transpose(1, 0, 2).reshape(DH, NH * H)
    ).astype(BF16)
    ipb = in_proj_b[:2 * H].reshape(2 * KO, P).T.astype(np.float32).copy()
    ipb[:, :KO] *= QSCALE                                # fold Q scaling
    vbv = in_proj_b[2 * H:].reshape(NH, P).T.astype(np.float32)
    owp = _feature_major(out_w.T.astype(np.float32)).astype(BF16)
    obv = out_b.reshape(KO, P).T.astype(np.float32)
    fcwp = _feature_major(fc_w.T.astype(np.float32)).astype(BF16)
    fcbv = fc_b.reshape(O // P, P).T.astype(np.float32)
    shared = dict(gw=gw, gb=np.ascontiguousarray(gb),
                  ip=ip, ipk2=ipk2, ipb=np.ascontiguousarray(ipb),
                  vb=np.ascontiguousarray(vbv), ow=owp,
                  ob=np.ascontiguousarray(obv), fcw=fcwp,
                  fcb=np.ascontiguousarray(fcbv))
    in_maps = []
    for c in range(NCORES):
        shard = x[c * BC:(c + 1) * BC]                   # [BC, S, H]
        xt = shard.transpose(2, 0, 1).reshape(H, BS)     # [H, BS]
        xt = _feature_major(xt).astype(BF16)             # [128, KO*BS]
        in_maps.append(dict(shared, xT=xt))
    return in_maps


def kernel(x, Wf, bf, Wi, bi, Wh, bh, in_proj_w, in_proj_b, out_w, out_b,
           fc_w, fc_b):
    from concourse.bass_utils import run_bass_kernel_spmd

    x, Wf, bf, Wi, bi, Wh, bh = (np.asarray(t) for t in
                                 (x, Wf, bf, Wi, bi, Wh, bh))
    in_proj_w, in_proj_b, out_w, out_b, fc_w, fc_b = (
        np.asarray(t) for t in (in_proj_w, in_proj_b, out_w, out_b,
                                fc_w, fc_b))
    if "nc" not in _CACHE:
        _CACHE["nc"] = _build_nc()
    nc = _CACHE["nc"]
    in_maps = _prep_inputs(x, Wf, bf, Wi, bi, Wh, bh, in_proj_w, in_proj_b,
                           out_w, out_b, fc_w, fc_b)
    res = run_bass_kernel_spmd(nc, in_maps, core_ids=list(range(NCORES)))
    _CACHE["last_results"] = res
    out = np.empty((B, O), np.float32)
    for c in range(NCORES):
        outT = res.results[c]["outT"]                    # [O, BC]
        for b in range(BC):
            out[c * BC + b] = outT[:, b]
    return out
